# revision 15
# baseline (speedup 1.0000x reference)
"""Multi-head attention (N=4, T=2048, D=512, H=8, dh=64) on 8 TRN2 NeuronCores.

Sharding: batch N (4) x head-group (2 groups of 4 heads) -> 8 cores.

v10: full-contraction AV + self-interleaved pipeline.
  - AV uses one full K=128 contraction chain per head per super-pair
    (2 serial matmuls per k-tile, ap-bound at the same wall rate as
    v5's 2-tile split) -> only 2 PSUM banks and 2 evacuation copies
    per super-pair, pure accumulation groups, no host halves-sum.
    (Mixed row-tile accumulation into one bank is rejected by HW --
    verified by tools/mergetest.py; full-contraction chains are the
    legal way to a 2-bank AV footprint.)
  - The 2-bank footprint frees PSUM (ring 3x2 + AV 2 = 8 banks) so
    each super-pair's AV interleaves into its OWN score rounds via a
    global work queue of (gate_round, closure) quanta, lagged LAG=3
    rounds behind the exp that produces its pt. The end-of-kernel AV
    tail collapses to ~3 rounds' worth.
  - exp rounds split 8/8 ScalarE (native Exp) / VectorE (Schraudolph
    bf16 = bitcast_i16(round(A*s + B))): at ~650ns/round pacing
    either engine would bottleneck at 10/16.
  - 6 scratch warmup matmuls keep the PE HAM clock at 8/8 through DMA
    staging; V projections for k-tiles 8-15 drain inside sp0's rounds.
Per k-tile round the head-lo score matmul (T0 = SBUF partitions 0-63)
and head-hi (T8 = 64-127) land in the same [128, 1024] PSUM ring slot
-> concurrent. One exp op per round covers both heads via the pt
layout [128, kt, head, 512]. Row 64 of each [65, 512] AV block is the
softmax denominator (ones column in vp); the HOST divides+transposes.
"""

import math

import ml_dtypes
import numpy as np

import concourse.bass as bass
import concourse.mybir as mybir
import concourse.tile as tile
from concourse import bacc
from concourse.bass_utils import run_bass_kernel_spmd

F32 = mybir.dt.float32
BF16 = mybir.dt.bfloat16
I16 = mybir.dt.int16
EXP = mybir.ActivationFunctionType.Exp
MULT = mybir.AluOpType.mult
ADD = mybir.AluOpType.add

N, T, D = 4, 2048, 512
HPC, DH = 4, 64
GC = HPC * DH
SCALE = 1.0 / math.sqrt(D)
QB = 512
NQB = T // QB            # 4
NKT = T // 128           # 16
KS = D // 128            # 4
OROW = DH + 1            # 65
ORWS = HPC * OROW        # 260

DVE_KTS = frozenset((0, 2, 4, 6, 8, 10, 12, 14))   # rounds exp'd by VectorE
SCH_C = 4.0
SCH_A = (128.0 / math.log(2.0)) * SCALE
SCH_B = 127.0 * 128.0 - SCH_C

N_WARM = 6
LAG = 3


def build():
    nc = bacc.Bacc("TRN2", target_bir_lowering=False, debug=False, num_devices=8)
    qT_in = nc.declare_dram_parameter("qT", [128, KS * T], BF16, isOutput=False)
    kT_in = nc.declare_dram_parameter("kT", [128, KS * T], BF16, isOutput=False)
    wq_in = nc.declare_dram_parameter("wq", [128, KS * GC], BF16, isOutput=False)
    wk_in = nc.declare_dram_parameter("wk", [128, KS * GC], BF16, isOutput=False)
    wv_in = nc.declare_dram_parameter("wv", [128, KS * GC], BF16, isOutput=False)
    oT_out = nc.declare_dram_parameter("oT65", [ORWS, T], F32, isOutput=True)

    with tile.TileContext(nc) as tc:
        with (
            tc.tile_pool(name="stage", bufs=2) as stage,
            tc.tile_pool(name="const", bufs=1) as const,
            tc.tile_pool(name="act", bufs=1) as actp,
            tc.tile_pool(name="ptc", bufs=3) as ptcp,
            tc.tile_pool(name="ost", bufs=8) as ostp,
            tc.tile_pool(name="ring", bufs=3, space="PSUM") as ring,  # 6 banks
            tc.tile_pool(name="psO", bufs=2, space="PSUM") as psO,    # 2 banks
        ):
            # ---- PE warmup on scratch (HAM at 8/8 before projections) ----
            scratch = const.tile([128, 640], BF16, tag="scr")
            nc.gpsimd.memset(scratch[:], 0.0)
            for w in range(N_WARM):
                wp = psO.tile([128, 512], F32, tag="O", name=f"warm{w}")
                nc.tensor.matmul(wp[:], scratch[:, 0:128], scratch[:, 128:640],
                                 start=True, stop=True)

            # ---- input staging: packed [128, KS, T] ----
            kin = stage.tile([128, KS, T], BF16, tag="kin")
            qin = stage.tile([128, KS, T], BF16, tag="qin")
            wv = const.tile([128, KS, GC], BF16, tag="wv")
            wk = const.tile([128, KS, GC], BF16, tag="wk")
            wq = const.tile([128, KS, GC], BF16, tag="wq")

            # 2KB-per-descriptor chunks (tb-pairs), K before Q: staging is
            # descriptor-rate-bound, so fatter per-partition contiguous
            # reads roughly double effective staging bandwidth.
            kT_r = kT_in.rearrange("p (s t) -> p s t", s=KS)
            qT_r = qT_in.rearrange("p (s t) -> p s t", s=KS)
            nc.sync.dma_start(wv[:], wv_in.rearrange("p (s c) -> p s c", s=KS))
            nc.sync.dma_start(wk[:], wk_in.rearrange("p (s c) -> p s c", s=KS))
            for s in range(KS):
                nc.sync.dma_start(kin[:, s, 0 : 2 * QB], kT_r[:, s, 0 : 2 * QB])
            nc.sync.dma_start(wq[:], wq_in.rearrange("p (s c) -> p s c", s=KS))
            for s in range(KS):
                nc.sync.dma_start(
                    kin[:, s, 2 * QB : 4 * QB], kT_r[:, s, 2 * QB : 4 * QB])
            for s in range(KS):
                nc.sync.dma_start(qin[:, s, 0 : 2 * QB], qT_r[:, s, 0 : 2 * QB])
            for s in range(KS):
                nc.sync.dma_start(
                    qin[:, s, 2 * QB : 4 * QB], qT_r[:, s, 2 * QB : 4 * QB])

            kT_att = [actp.tile([128, T], BF16, tag=f"ka{d}", name=f"ka{d}")
                      for d in range(2)]
            qT_att = [actp.tile([128, T], BF16, tag=f"qa{d}", name=f"qa{d}")
                      for d in range(2)]

            vp = const.tile([128, NKT, HPC, OROW], BF16, tag="vp")
            ones_f32 = const.tile([128, NKT * HPC], F32, tag="ones")
            nc.gpsimd.memset(ones_f32[:], 1.0)
            nc.vector.tensor_copy(
                vp[:, :, :, DH : DH + 1],
                ones_f32[:].rearrange("p (a b) -> p a b", b=HPC).unsqueeze(3))

            # ---- projections: full-array K=128 ----
            def emit_kqproj(which, tb):
                w, src, dst = (
                    (wk, kin, kT_att) if which == "k" else (wq, qin, qT_att))
                cols = slice(tb * QB, (tb + 1) * QB)
                for dt2 in range(2):
                    ps = psO.tile([128, QB], F32, tag="O",
                                  name=f"{which}p{tb}_{dt2}")
                    for s in range(KS):
                        nc.tensor.matmul(
                            ps[:], w[:, s, dt2 * 128 : (dt2 + 1) * 128],
                            src[:, s, cols], start=(s == 0), stop=(s == KS - 1))
                    nc.vector.tensor_copy(dst[dt2][:, cols], ps[:])

            def emit_vproj(tt):
                ps = psO.tile([128, QB], F32, tag="O", name=f"vp{tt}")
                for s in range(KS):
                    nc.tensor.matmul(
                        ps[:, 0:GC], kin[:, s, tt * 128 : (tt + 1) * 128],
                        wv[:, s, :], start=(s == 0), stop=(s == KS - 1))
                nc.vector.tensor_copy(
                    vp[:, tt, :, 0:DH],
                    ps[:, 0:GC].rearrange("p (h d) -> p h d", d=DH))

            for tt in range(4):
                emit_vproj(tt)
            for tb in range(NQB):
                emit_kqproj("k", tb)
                emit_kqproj("q", tb)
            for tt in range(4, NKT):
                emit_vproj(tt)

            # ---- attention ----
            # global work queue: (gate_round, closure); quanta drain once the
            # absolute round index >= gate_round, ceil-paced per super-pair.
            queue = []

            def emit_out(t2, qb, po, which, spi):
                hp = 2 * t2 + (0 if which == "lo" else 1)
                st = ostp.tile([128, QB], F32, tag="ost", name=f"o{which}{spi}")
                if which == "lo":
                    nc.scalar.copy(st[0:OROW, :], po[0:OROW, :])
                else:
                    nc.vector.tensor_copy(st[0:OROW, :], po[0:OROW, :])
                nc.sync.dma_start(
                    oT_out[hp * OROW : (hp + 1) * OROW,
                           qb * QB : (qb + 1) * QB],
                    st[0:OROW, :])

            def queue_av(spi, t2, qb, pt):
                """Full-contraction AV: per k-tile, one K=128 matmul per head
                accumulating into that head's bank (pure groups)."""
                hp_lo, hp_hi = 2 * t2, 2 * t2 + 1
                hold = {}

                def slot(kt):
                    def go():
                        if "lo" not in hold:
                            hold["lo"] = psO.tile([128, QB], F32, tag="O",
                                                  name=f"po_lo{spi}")
                            hold["hi"] = psO.tile([128, QB], F32, tag="O",
                                                  name=f"po_hi{spi}")
                        nc.tensor.matmul(
                            hold["lo"][0:OROW], vp[:, kt, hp_lo, :],
                            pt[:, kt, 0, :],
                            start=(kt == 0), stop=(kt == NKT - 1))
                        nc.tensor.matmul(
                            hold["hi"][0:OROW], vp[:, kt, hp_hi, :],
                            pt[:, kt, 1, :],
                            start=(kt == 0), stop=(kt == NKT - 1))
                    return go

                base = spi * NKT
                for kt in range(NKT):
                    queue.append((base + kt + LAG, slot(kt)))
                queue.append((base + NKT - 1 + LAG,
                              lambda: emit_out(t2, qb, hold["lo"], "lo", spi)))
                queue.append((base + NKT - 1 + LAG,
                              lambda: emit_out(t2, qb, hold["hi"], "hi", spi)))

            def emit_scores(spi, t2, qb, pt):
                q_lo = qT_att[t2][0:DH, qb * QB : (qb + 1) * QB]
                q_hi = qT_att[t2][DH:128, qb * QB : (qb + 1) * QB]
                for kt in range(NKT):
                    R = spi * NKT + kt
                    # drain in 2-round bursts: full-contraction AV matmuls
                    # occupy all PE row groups, so the score right after one
                    # loses its 2-tile pairing -- pairing the score rounds
                    # amortizes that break, keeps the exp engines strictly
                    # alternating (no double-queue within a burst), and
                    # ring=3 gives each exp ~2 rounds of latency slack.
                    if kt % 2 == 0:
                        nq = -(-len(queue) * 2 // (NKT - kt))  # ceil pacing
                        while queue and nq > 0 and queue[0][0] <= R:
                            queue.pop(0)[1]()
                            nq -= 1
                    sl = ring.tile([128, 2 * QB], F32, tag="R", name=f"sc{kt}")
                    nc.tensor.matmul(
                        sl[:, 0:QB],
                        kT_att[t2][0:DH, kt * 128 : (kt + 1) * 128],
                        q_lo, start=True, stop=True)
                    nc.tensor.matmul(
                        sl[:, QB : 2 * QB],
                        kT_att[t2][DH:128, kt * 128 : (kt + 1) * 128],
                        q_hi, start=True, stop=True)
                    if kt in DVE_KTS:
                        nc.vector.tensor_scalar(
                            pt[:, kt, :, :].bitcast(I16), sl[:],
                            SCH_A, SCH_B, MULT, ADD)
                    else:
                        nc.scalar.activation(pt[:, kt, :, :], sl[:], EXP,
                                             scale=SCALE)

            sps = [(qb, t2) for qb in range(NQB) for t2 in range(2)]
            for spi, (qb, t2) in enumerate(sps):
                pt = ptcp.tile([128, NKT, 2, QB], BF16, tag="ptc", name="ptc")
                queue_av(spi, t2, qb, pt)
                emit_scores(spi, t2, qb, pt)
            for _, fn in queue:
                fn()
            queue.clear()

    nc.compile()
    return nc


_NC = None


def _get_nc():
    global _NC
    if _NC is None:
        _NC = build()
    return _NC


def _prep_w(W, cols):
    w = W[:, cols].astype(ml_dtypes.bfloat16)           # [512, 256]
    w = w.reshape(KS, 128, GC).transpose(1, 0, 2)       # [128, KS, GC]
    return np.ascontiguousarray(w.reshape(128, KS * GC))


def _prep_x(x):
    # [T, 512] -> [128, KS*T] with xp[p, s*T + t] = x[t, s*128 + p]
    xt = x.T.astype(ml_dtypes.bfloat16)                 # [512, T]
    xt = xt.reshape(KS, 128, T).transpose(1, 0, 2)      # [128, KS, T]
    return np.ascontiguousarray(xt.reshape(128, KS * T))


def run(query, key, W_query, W_key, W_value, trace=False):
    nc = _get_nc()
    query = np.asarray(query, dtype=np.float32)
    key = np.asarray(key, dtype=np.float32)
    W_query = np.asarray(W_query, dtype=np.float32)
    W_key = np.asarray(W_key, dtype=np.float32)
    W_value = np.asarray(W_value, dtype=np.float32)

    in_maps = []
    for c in range(8):
        n, g = c // 2, c % 2
        cols = slice(g * GC, (g + 1) * GC)
        in_maps.append(
            {
                "qT": _prep_x(query[n]),
                "kT": _prep_x(key[n]),
                "wq": _prep_w(W_query, cols),
                "wk": _prep_w(W_key, cols),
                "wv": _prep_w(W_value, cols),
            }
        )
    res = run_bass_kernel_spmd(nc, in_maps, core_ids=list(range(8)), trace=trace)
    out = np.empty((N, T, D), dtype=np.float32)
    for c in range(8):
        n, g = c // 2, c % 2
        full = res.results[c]["oT65"]  # [260, 2048]
        for hp in range(HPC):
            blk = full[hp * OROW : (hp + 1) * OROW]
            out[n, :, g * GC + hp * DH : g * GC + (hp + 1) * DH] = (
                blk[0:DH] / blk[DH : DH + 1]
            ).T
    return out, res


def kernel(query, key, W_query, W_key, W_value):
    out, _ = run(query, key, W_query, W_key, W_value, trace=False)
    return out


# revision 17
# speedup vs baseline: 1.0099x; 1.0099x over previous
"""Multi-head attention (N=4, T=2048, D=512, H=8, dh=64) on 8 TRN2 NeuronCores.

Sharding: batch N (4) x head-group (2 groups of 4 heads) -> 8 cores.

v10: full-contraction AV + self-interleaved pipeline.
  - AV uses one full K=128 contraction chain per head per super-pair
    (2 serial matmuls per k-tile, ap-bound at the same wall rate as
    v5's 2-tile split) -> only 2 PSUM banks and 2 evacuation copies
    per super-pair, pure accumulation groups, no host halves-sum.
    (Mixed row-tile accumulation into one bank is rejected by HW --
    verified by tools/mergetest.py; full-contraction chains are the
    legal way to a 2-bank AV footprint.)
  - The 2-bank footprint frees PSUM (ring 3x2 + AV 2 = 8 banks) so
    each super-pair's AV interleaves into its OWN score rounds via a
    global work queue of (gate_round, closure) quanta, lagged LAG=3
    rounds behind the exp that produces its pt. The end-of-kernel AV
    tail collapses to ~3 rounds' worth.
  - exp rounds split 8/8 ScalarE (native Exp) / VectorE (Schraudolph
    bf16 = bitcast_i16(round(A*s + B))): at ~650ns/round pacing
    either engine would bottleneck at 10/16.
  - 6 scratch warmup matmuls keep the PE HAM clock at 8/8 through DMA
    staging; V projections for k-tiles 8-15 drain inside sp0's rounds.
Per k-tile round the head-lo score matmul (T0 = SBUF partitions 0-63)
and head-hi (T8 = 64-127) land in the same [128, 1024] PSUM ring slot
-> concurrent. One exp op per round covers both heads via the pt
layout [128, kt, head, 512]. Row 64 of each [65, 512] AV block is the
softmax denominator (ones column in vp); the HOST divides+transposes.
"""

import math

import ml_dtypes
import numpy as np

import concourse.bass as bass
import concourse.mybir as mybir
import concourse.tile as tile
from concourse import bacc
from concourse.bass_utils import run_bass_kernel_spmd

F32 = mybir.dt.float32
BF16 = mybir.dt.bfloat16
I16 = mybir.dt.int16
EXP = mybir.ActivationFunctionType.Exp
MULT = mybir.AluOpType.mult
ADD = mybir.AluOpType.add

N, T, D = 4, 2048, 512
HPC, DH = 4, 64
GC = HPC * DH
SCALE = 1.0 / math.sqrt(D)
QB = 512
NQB = T // QB            # 4
NKT = T // 128           # 16
KS = D // 128            # 4
OROW = DH + 1            # 65
ORWS = HPC * OROW        # 260

DVE_KTS = frozenset((1, 3, 5, 7, 9, 11, 13, 15))   # rounds exp'd by VectorE
SCH_C = 4.0
SCH_A = (128.0 / math.log(2.0)) * SCALE
SCH_B = 127.0 * 128.0 - SCH_C

N_WARM = 6
LAG = 3


def build():
    nc = bacc.Bacc("TRN2", target_bir_lowering=False, debug=False, num_devices=8)
    qT_in = nc.declare_dram_parameter("qT", [128, KS * T], BF16, isOutput=False)
    kT_in = nc.declare_dram_parameter("kT", [128, KS * T], BF16, isOutput=False)
    wq_in = nc.declare_dram_parameter("wq", [128, KS * GC], BF16, isOutput=False)
    wk_in = nc.declare_dram_parameter("wk", [128, KS * GC], BF16, isOutput=False)
    wv_in = nc.declare_dram_parameter("wv", [128, KS * GC], BF16, isOutput=False)
    oT_out = nc.declare_dram_parameter("oT65", [ORWS, T], F32, isOutput=True)

    with tile.TileContext(nc) as tc:
        with (
            tc.tile_pool(name="stage", bufs=2) as stage,
            tc.tile_pool(name="const", bufs=1) as const,
            tc.tile_pool(name="act", bufs=1) as actp,
            tc.tile_pool(name="ptc", bufs=3) as ptcp,
            tc.tile_pool(name="ost", bufs=8) as ostp,
            tc.tile_pool(name="ring", bufs=3, space="PSUM") as ring,  # 6 banks
            tc.tile_pool(name="psO", bufs=2, space="PSUM") as psO,    # 2 banks
        ):
            # ---- PE warmup on scratch (HAM at 8/8 before projections) ----
            scratch = const.tile([128, 640], BF16, tag="scr")
            nc.gpsimd.memset(scratch[:], 0.0)
            for w in range(N_WARM):
                wp = psO.tile([128, 512], F32, tag="O", name=f"warm{w}")
                nc.tensor.matmul(wp[:], scratch[:, 0:128], scratch[:, 128:640],
                                 start=True, stop=True)

            # ---- input staging: packed [128, KS, T] ----
            kin = stage.tile([128, KS, T], BF16, tag="kin")
            qin = stage.tile([128, KS, T], BF16, tag="qin")
            wv = const.tile([128, KS, GC], BF16, tag="wv")
            wk = const.tile([128, KS, GC], BF16, tag="wk")
            wq = const.tile([128, KS, GC], BF16, tag="wq")

            kT_r = kT_in.rearrange("p (s t) -> p s t", s=KS)
            qT_r = qT_in.rearrange("p (s t) -> p s t", s=KS)
            nc.sync.dma_start(wv[:], wv_in.rearrange("p (s c) -> p s c", s=KS))
            nc.sync.dma_start(wk[:], wk_in.rearrange("p (s c) -> p s c", s=KS))
            for s in range(KS):
                nc.sync.dma_start(kin[:, s, 0:QB], kT_r[:, s, 0:QB])
            nc.sync.dma_start(wq[:], wq_in.rearrange("p (s c) -> p s c", s=KS))
            for s in range(KS):
                nc.sync.dma_start(qin[:, s, 0:QB], qT_r[:, s, 0:QB])
            for tb in range(1, NQB):
                for s in range(KS):
                    nc.sync.dma_start(
                        kin[:, s, tb * QB : (tb + 1) * QB],
                        kT_r[:, s, tb * QB : (tb + 1) * QB])
                for s in range(KS):
                    nc.sync.dma_start(
                        qin[:, s, tb * QB : (tb + 1) * QB],
                        qT_r[:, s, tb * QB : (tb + 1) * QB])

            kT_att = [actp.tile([128, T], BF16, tag=f"ka{d}", name=f"ka{d}")
                      for d in range(2)]
            qT_att = [actp.tile([128, T], BF16, tag=f"qa{d}", name=f"qa{d}")
                      for d in range(2)]

            vp = const.tile([128, NKT, HPC, OROW], BF16, tag="vp")
            ones_f32 = const.tile([128, NKT * HPC], F32, tag="ones")
            nc.gpsimd.memset(ones_f32[:], 1.0)
            nc.vector.tensor_copy(
                vp[:, :, :, DH : DH + 1],
                ones_f32[:].rearrange("p (a b) -> p a b", b=HPC).unsqueeze(3))

            # ---- projections: full-array K=128 ----
            def emit_kqproj(which, tb):
                w, src, dst = (
                    (wk, kin, kT_att) if which == "k" else (wq, qin, qT_att))
                cols = slice(tb * QB, (tb + 1) * QB)
                for dt2 in range(2):
                    ps = psO.tile([128, QB], F32, tag="O",
                                  name=f"{which}p{tb}_{dt2}")
                    for s in range(KS):
                        nc.tensor.matmul(
                            ps[:], w[:, s, dt2 * 128 : (dt2 + 1) * 128],
                            src[:, s, cols], start=(s == 0), stop=(s == KS - 1))
                    nc.vector.tensor_copy(dst[dt2][:, cols], ps[:])

            def emit_vproj(tt):
                ps = psO.tile([128, QB], F32, tag="O", name=f"vp{tt}")
                for s in range(KS):
                    nc.tensor.matmul(
                        ps[:, 0:GC], kin[:, s, tt * 128 : (tt + 1) * 128],
                        wv[:, s, :], start=(s == 0), stop=(s == KS - 1))
                nc.vector.tensor_copy(
                    vp[:, tt, :, 0:DH],
                    ps[:, 0:GC].rearrange("p (h d) -> p h d", d=DH))

            for tt in range(4):
                emit_vproj(tt)
            for tb in range(NQB):
                emit_kqproj("k", tb)
                emit_kqproj("q", tb)
            for tt in range(4, NKT):
                emit_vproj(tt)

            # ---- attention ----
            # global work queue: (gate_round, closure); quanta drain once the
            # absolute round index >= gate_round, ceil-paced per super-pair.
            queue = []

            def emit_out(t2, qb, po, which, spi):
                hp = 2 * t2 + (0 if which == "lo" else 1)
                st = ostp.tile([128, QB], F32, tag="ost", name=f"o{which}{spi}")
                if which == "lo":
                    nc.scalar.copy(st[0:OROW, :], po[0:OROW, :])
                else:
                    nc.vector.tensor_copy(st[0:OROW, :], po[0:OROW, :])
                nc.sync.dma_start(
                    oT_out[hp * OROW : (hp + 1) * OROW,
                           qb * QB : (qb + 1) * QB],
                    st[0:OROW, :])

            def queue_av(spi, t2, qb, pt):
                """Full-contraction AV: per k-tile, one K=128 matmul per head
                accumulating into that head's bank (pure groups)."""
                hp_lo, hp_hi = 2 * t2, 2 * t2 + 1
                hold = {}

                def slot(kt):
                    def go():
                        if "lo" not in hold:
                            hold["lo"] = psO.tile([128, QB], F32, tag="O",
                                                  name=f"po_lo{spi}")
                            hold["hi"] = psO.tile([128, QB], F32, tag="O",
                                                  name=f"po_hi{spi}")
                        nc.tensor.matmul(
                            hold["lo"][0:OROW], vp[:, kt, hp_lo, :],
                            pt[:, kt, 0, :],
                            start=(kt == 0), stop=(kt == NKT - 1))
                        nc.tensor.matmul(
                            hold["hi"][0:OROW], vp[:, kt, hp_hi, :],
                            pt[:, kt, 1, :],
                            start=(kt == 0), stop=(kt == NKT - 1))
                    return go

                base = spi * NKT
                for kt in range(NKT):
                    queue.append((base + kt + LAG, slot(kt)))
                queue.append((base + NKT - 1 + LAG,
                              lambda: emit_out(t2, qb, hold["lo"], "lo", spi)))
                queue.append((base + NKT - 1 + LAG,
                              lambda: emit_out(t2, qb, hold["hi"], "hi", spi)))

            def emit_scores(spi, t2, qb, pt):
                q_lo = qT_att[t2][0:DH, qb * QB : (qb + 1) * QB]
                q_hi = qT_att[t2][DH:128, qb * QB : (qb + 1) * QB]
                for kt in range(NKT):
                    R = spi * NKT + kt
                    # drain in 2-round bursts: full-contraction AV matmuls
                    # occupy all PE row groups, so the score right after one
                    # loses its 2-tile pairing -- pairing the score rounds
                    # amortizes that break, keeps the exp engines strictly
                    # alternating (no double-queue within a burst), and
                    # ring=3 gives each exp ~2 rounds of latency slack.
                    if kt % 2 == 0:
                        nq = -(-len(queue) * 2 // (NKT - kt))  # ceil pacing
                        while queue and nq > 0 and queue[0][0] <= R:
                            queue.pop(0)[1]()
                            nq -= 1
                    sl = ring.tile([128, 2 * QB], F32, tag="R", name=f"sc{kt}")
                    nc.tensor.matmul(
                        sl[:, 0:QB],
                        kT_att[t2][0:DH, kt * 128 : (kt + 1) * 128],
                        q_lo, start=True, stop=True)
                    nc.tensor.matmul(
                        sl[:, QB : 2 * QB],
                        kT_att[t2][DH:128, kt * 128 : (kt + 1) * 128],
                        q_hi, start=True, stop=True)
                    if kt in DVE_KTS:
                        nc.vector.tensor_scalar(
                            pt[:, kt, :, :].bitcast(I16), sl[:],
                            SCH_A, SCH_B, MULT, ADD)
                    else:
                        nc.scalar.activation(pt[:, kt, :, :], sl[:], EXP,
                                             scale=SCALE)

            sps = [(qb, t2) for qb in range(NQB) for t2 in range(2)]
            for spi, (qb, t2) in enumerate(sps):
                pt = ptcp.tile([128, NKT, 2, QB], BF16, tag="ptc", name="ptc")
                queue_av(spi, t2, qb, pt)
                emit_scores(spi, t2, qb, pt)
            for _, fn in queue:
                fn()
            queue.clear()

    nc.compile()
    return nc


_NC = None


def _get_nc():
    global _NC
    if _NC is None:
        _NC = build()
    return _NC


def _prep_w(W, cols):
    w = W[:, cols].astype(ml_dtypes.bfloat16)           # [512, 256]
    w = w.reshape(KS, 128, GC).transpose(1, 0, 2)       # [128, KS, GC]
    return np.ascontiguousarray(w.reshape(128, KS * GC))


def _prep_x(x):
    # [T, 512] -> [128, KS*T] with xp[p, s*T + t] = x[t, s*128 + p]
    xt = x.T.astype(ml_dtypes.bfloat16)                 # [512, T]
    xt = xt.reshape(KS, 128, T).transpose(1, 0, 2)      # [128, KS, T]
    return np.ascontiguousarray(xt.reshape(128, KS * T))


def run(query, key, W_query, W_key, W_value, trace=False):
    nc = _get_nc()
    query = np.asarray(query, dtype=np.float32)
    key = np.asarray(key, dtype=np.float32)
    W_query = np.asarray(W_query, dtype=np.float32)
    W_key = np.asarray(W_key, dtype=np.float32)
    W_value = np.asarray(W_value, dtype=np.float32)

    in_maps = []
    for c in range(8):
        n, g = c // 2, c % 2
        cols = slice(g * GC, (g + 1) * GC)
        in_maps.append(
            {
                "qT": _prep_x(query[n]),
                "kT": _prep_x(key[n]),
                "wq": _prep_w(W_query, cols),
                "wk": _prep_w(W_key, cols),
                "wv": _prep_w(W_value, cols),
            }
        )
    res = run_bass_kernel_spmd(nc, in_maps, core_ids=list(range(8)), trace=trace)
    out = np.empty((N, T, D), dtype=np.float32)
    for c in range(8):
        n, g = c // 2, c % 2
        full = res.results[c]["oT65"]  # [260, 2048]
        for hp in range(HPC):
            blk = full[hp * OROW : (hp + 1) * OROW]
            out[n, :, g * GC + hp * DH : g * GC + (hp + 1) * DH] = (
                blk[0:DH] / blk[DH : DH + 1]
            ).T
    return out, res


def kernel(query, key, W_query, W_key, W_value):
    out, _ = run(query, key, W_query, W_key, W_value, trace=False)
    return out


# revision 19
# speedup vs baseline: 1.0392x; 1.0291x over previous
"""Multi-head attention (N=4, T=2048, D=512, H=8, dh=64) on 8 TRN2 NeuronCores.

Sharding: batch N (4) x head-group (2 groups of 4 heads) -> 8 cores.

v10: full-contraction AV + self-interleaved pipeline.
  - AV uses one full K=128 contraction chain per head per super-pair
    (2 serial matmuls per k-tile, ap-bound at the same wall rate as
    v5's 2-tile split) -> only 2 PSUM banks and 2 evacuation copies
    per super-pair, pure accumulation groups, no host halves-sum.
    (Mixed row-tile accumulation into one bank is rejected by HW --
    verified by tools/mergetest.py; full-contraction chains are the
    legal way to a 2-bank AV footprint.)
  - The 2-bank footprint frees PSUM (ring 3x2 + AV 2 = 8 banks) so
    each super-pair's AV interleaves into its OWN score rounds via a
    global work queue of (gate_round, closure) quanta, lagged LAG=3
    rounds behind the exp that produces its pt. The end-of-kernel AV
    tail collapses to ~3 rounds' worth.
  - exp rounds split 8/8 ScalarE (native Exp) / VectorE (Schraudolph
    bf16 = bitcast_i16(round(A*s + B))): at ~650ns/round pacing
    either engine would bottleneck at 10/16.
  - 6 scratch warmup matmuls keep the PE HAM clock at 8/8 through DMA
    staging; V projections for k-tiles 8-15 drain inside sp0's rounds.
Per k-tile round the head-lo score matmul (T0 = SBUF partitions 0-63)
and head-hi (T8 = 64-127) land in the same [128, 1024] PSUM ring slot
-> concurrent. One exp op per round covers both heads via the pt
layout [128, kt, head, 512]. Row 64 of each [65, 512] AV block is the
softmax denominator (ones column in vp); the HOST divides+transposes.
"""

import math

import ml_dtypes
import numpy as np

import concourse.bass as bass
import concourse.mybir as mybir
import concourse.tile as tile
from concourse import bacc
from concourse.bass_utils import run_bass_kernel_spmd

F32 = mybir.dt.float32
BF16 = mybir.dt.bfloat16
I16 = mybir.dt.int16
EXP = mybir.ActivationFunctionType.Exp
MULT = mybir.AluOpType.mult
ADD = mybir.AluOpType.add

N, T, D = 4, 2048, 512
HPC, DH = 4, 64
GC = HPC * DH
SCALE = 1.0 / math.sqrt(D)
QB = 512
NQB = T // QB            # 4
NKT = T // 128           # 16
KS = D // 128            # 4
OROW = DH + 1            # 65
ORWS = HPC * OROW        # 260

# rounds exp'd by VectorE: odd k-tiles in steady state (so the tight
# ring dependency lands on the faster ScalarE), evens during sp0 (the
# prologue/sp0 schedule packs better that way, measured)
def _dve_round(spi, kt):
    return kt % 2 == (0 if spi == 0 else 1)
SCH_C = 4.0
SCH_A = (128.0 / math.log(2.0)) * SCALE
SCH_B = 127.0 * 128.0 - SCH_C

N_WARM = 6
LAG = 3


def build():
    nc = bacc.Bacc("TRN2", target_bir_lowering=False, debug=False, num_devices=8)
    qT_in = nc.declare_dram_parameter("qT", [128, KS * T], BF16, isOutput=False)
    kT_in = nc.declare_dram_parameter("kT", [128, KS * T], BF16, isOutput=False)
    wq_in = nc.declare_dram_parameter("wq", [128, KS * GC], BF16, isOutput=False)
    wk_in = nc.declare_dram_parameter("wk", [128, KS * GC], BF16, isOutput=False)
    wv_in = nc.declare_dram_parameter("wv", [128, KS * GC], BF16, isOutput=False)
    oT_out = nc.declare_dram_parameter("oT65", [ORWS, T], F32, isOutput=True)

    with tile.TileContext(nc) as tc:
        with (
            tc.tile_pool(name="stage", bufs=2) as stage,
            tc.tile_pool(name="const", bufs=1) as const,
            tc.tile_pool(name="act", bufs=1) as actp,
            tc.tile_pool(name="ptc", bufs=3) as ptcp,
            tc.tile_pool(name="ost", bufs=8) as ostp,
            tc.tile_pool(name="ring", bufs=3, space="PSUM") as ring,  # 6 banks
            tc.tile_pool(name="psO", bufs=2, space="PSUM") as psO,    # 2 banks
        ):
            # ---- PE warmup on scratch (HAM at 8/8 before projections) ----
            scratch = const.tile([128, 640], BF16, tag="scr")
            nc.gpsimd.memset(scratch[:], 0.0)
            for w in range(N_WARM):
                wp = psO.tile([128, 512], F32, tag="O", name=f"warm{w}")
                nc.tensor.matmul(wp[:], scratch[:, 0:128], scratch[:, 128:640],
                                 start=True, stop=True)

            # ---- input staging: packed [128, KS, T] ----
            kin = stage.tile([128, KS, T], BF16, tag="kin")
            qin = stage.tile([128, KS, T], BF16, tag="qin")
            wv = const.tile([128, KS, GC], BF16, tag="wv")
            wk = const.tile([128, KS, GC], BF16, tag="wk")
            wq = const.tile([128, KS, GC], BF16, tag="wq")

            kT_r = kT_in.rearrange("p (s t) -> p s t", s=KS)
            qT_r = qT_in.rearrange("p (s t) -> p s t", s=KS)
            nc.sync.dma_start(wv[:], wv_in.rearrange("p (s c) -> p s c", s=KS))
            nc.sync.dma_start(wk[:], wk_in.rearrange("p (s c) -> p s c", s=KS))
            for s in range(KS):
                nc.sync.dma_start(kin[:, s, 0:QB], kT_r[:, s, 0:QB])
            nc.sync.dma_start(wq[:], wq_in.rearrange("p (s c) -> p s c", s=KS))
            for s in range(KS):
                nc.sync.dma_start(qin[:, s, 0:QB], qT_r[:, s, 0:QB])
            for tb in range(1, NQB):
                for s in range(KS):
                    nc.sync.dma_start(
                        kin[:, s, tb * QB : (tb + 1) * QB],
                        kT_r[:, s, tb * QB : (tb + 1) * QB])
                for s in range(KS):
                    nc.sync.dma_start(
                        qin[:, s, tb * QB : (tb + 1) * QB],
                        qT_r[:, s, tb * QB : (tb + 1) * QB])

            kT_att = [actp.tile([128, T], BF16, tag=f"ka{d}", name=f"ka{d}")
                      for d in range(2)]
            qT_att = [actp.tile([128, T], BF16, tag=f"qa{d}", name=f"qa{d}")
                      for d in range(2)]

            vp = const.tile([128, NKT, HPC, OROW], BF16, tag="vp")
            ones_f32 = const.tile([128, NKT * HPC], F32, tag="ones")
            nc.gpsimd.memset(ones_f32[:], 1.0)
            nc.vector.tensor_copy(
                vp[:, :, :, DH : DH + 1],
                ones_f32[:].rearrange("p (a b) -> p a b", b=HPC).unsqueeze(3))

            # ---- projections: full-array K=128 ----
            def emit_kqproj(which, tb):
                w, src, dst = (
                    (wk, kin, kT_att) if which == "k" else (wq, qin, qT_att))
                cols = slice(tb * QB, (tb + 1) * QB)
                for dt2 in range(2):
                    ps = psO.tile([128, QB], F32, tag="O",
                                  name=f"{which}p{tb}_{dt2}")
                    for s in range(KS):
                        nc.tensor.matmul(
                            ps[:], w[:, s, dt2 * 128 : (dt2 + 1) * 128],
                            src[:, s, cols], start=(s == 0), stop=(s == KS - 1))
                    nc.vector.tensor_copy(dst[dt2][:, cols], ps[:])

            def emit_vproj(tt):
                ps = psO.tile([128, QB], F32, tag="O", name=f"vp{tt}")
                for s in range(KS):
                    nc.tensor.matmul(
                        ps[:, 0:GC], kin[:, s, tt * 128 : (tt + 1) * 128],
                        wv[:, s, :], start=(s == 0), stop=(s == KS - 1))
                nc.vector.tensor_copy(
                    vp[:, tt, :, 0:DH],
                    ps[:, 0:GC].rearrange("p (h d) -> p h d", d=DH))

            for tt in range(4):
                emit_vproj(tt)
            for tb in range(NQB):
                emit_kqproj("k", tb)
                emit_kqproj("q", tb)
            for tt in range(4, NKT):
                emit_vproj(tt)

            # ---- attention ----
            # global work queue: (gate_round, closure); quanta drain once the
            # absolute round index >= gate_round, ceil-paced per super-pair.
            queue = []

            def emit_out(t2, qb, po, which, spi):
                hp = 2 * t2 + (0 if which == "lo" else 1)
                st = ostp.tile([128, QB], F32, tag="ost", name=f"o{which}{spi}")
                if which == "lo":
                    nc.scalar.copy(st[0:OROW, :], po[0:OROW, :])
                else:
                    nc.vector.tensor_copy(st[0:OROW, :], po[0:OROW, :])
                nc.sync.dma_start(
                    oT_out[hp * OROW : (hp + 1) * OROW,
                           qb * QB : (qb + 1) * QB],
                    st[0:OROW, :])

            def queue_av(spi, t2, qb, pt):
                """Full-contraction AV: per k-tile, one K=128 matmul per head
                accumulating into that head's bank (pure groups)."""
                hp_lo, hp_hi = 2 * t2, 2 * t2 + 1
                hold = {}

                def slot(kt):
                    def go():
                        if "lo" not in hold:
                            hold["lo"] = psO.tile([128, QB], F32, tag="O",
                                                  name=f"po_lo{spi}")
                            hold["hi"] = psO.tile([128, QB], F32, tag="O",
                                                  name=f"po_hi{spi}")
                        nc.tensor.matmul(
                            hold["lo"][0:OROW], vp[:, kt, hp_lo, :],
                            pt[:, kt, 0, :],
                            start=(kt == 0), stop=(kt == NKT - 1))
                        nc.tensor.matmul(
                            hold["hi"][0:OROW], vp[:, kt, hp_hi, :],
                            pt[:, kt, 1, :],
                            start=(kt == 0), stop=(kt == NKT - 1))
                    return go

                base = spi * NKT
                for kt in range(NKT):
                    queue.append((base + kt + LAG, slot(kt)))
                queue.append((base + NKT - 1 + LAG,
                              lambda: emit_out(t2, qb, hold["lo"], "lo", spi)))
                queue.append((base + NKT - 1 + LAG,
                              lambda: emit_out(t2, qb, hold["hi"], "hi", spi)))

            def emit_scores(spi, t2, qb, pt):
                q_lo = qT_att[t2][0:DH, qb * QB : (qb + 1) * QB]
                q_hi = qT_att[t2][DH:128, qb * QB : (qb + 1) * QB]
                for kt in range(NKT):
                    R = spi * NKT + kt
                    # drain in 2-round bursts: full-contraction AV matmuls
                    # occupy all PE row groups, so the score right after one
                    # loses its 2-tile pairing -- pairing the score rounds
                    # amortizes that break, keeps the exp engines strictly
                    # alternating (no double-queue within a burst), and
                    # ring=3 gives each exp ~2 rounds of latency slack.
                    if kt % 2 == 0:
                        nq = -(-len(queue) * 2 // (NKT - kt))  # ceil pacing
                        while queue and nq > 0 and queue[0][0] <= R:
                            queue.pop(0)[1]()
                            nq -= 1
                    sl = ring.tile([128, 2 * QB], F32, tag="R", name=f"sc{kt}")
                    nc.tensor.matmul(
                        sl[:, 0:QB],
                        kT_att[t2][0:DH, kt * 128 : (kt + 1) * 128],
                        q_lo, start=True, stop=True)
                    nc.tensor.matmul(
                        sl[:, QB : 2 * QB],
                        kT_att[t2][DH:128, kt * 128 : (kt + 1) * 128],
                        q_hi, start=True, stop=True)
                    if _dve_round(spi, kt):
                        nc.vector.tensor_scalar(
                            pt[:, kt, :, :].bitcast(I16), sl[:],
                            SCH_A, SCH_B, MULT, ADD)
                    else:
                        nc.scalar.activation(pt[:, kt, :, :], sl[:], EXP,
                                             scale=SCALE)

            sps = [(qb, t2) for qb in range(NQB) for t2 in range(2)]
            for spi, (qb, t2) in enumerate(sps):
                pt = ptcp.tile([128, NKT, 2, QB], BF16, tag="ptc", name="ptc")
                queue_av(spi, t2, qb, pt)
                emit_scores(spi, t2, qb, pt)
            for _, fn in queue:
                fn()
            queue.clear()

    nc.compile()
    return nc


_NC = None


def _get_nc():
    global _NC
    if _NC is None:
        _NC = build()
    return _NC


def _prep_w(W, cols):
    w = W[:, cols].astype(ml_dtypes.bfloat16)           # [512, 256]
    w = w.reshape(KS, 128, GC).transpose(1, 0, 2)       # [128, KS, GC]
    return np.ascontiguousarray(w.reshape(128, KS * GC))


def _prep_x(x):
    # [T, 512] -> [128, KS*T] with xp[p, s*T + t] = x[t, s*128 + p]
    xt = x.T.astype(ml_dtypes.bfloat16)                 # [512, T]
    xt = xt.reshape(KS, 128, T).transpose(1, 0, 2)      # [128, KS, T]
    return np.ascontiguousarray(xt.reshape(128, KS * T))


def run(query, key, W_query, W_key, W_value, trace=False):
    nc = _get_nc()
    query = np.asarray(query, dtype=np.float32)
    key = np.asarray(key, dtype=np.float32)
    W_query = np.asarray(W_query, dtype=np.float32)
    W_key = np.asarray(W_key, dtype=np.float32)
    W_value = np.asarray(W_value, dtype=np.float32)

    in_maps = []
    for c in range(8):
        n, g = c // 2, c % 2
        cols = slice(g * GC, (g + 1) * GC)
        in_maps.append(
            {
                "qT": _prep_x(query[n]),
                "kT": _prep_x(key[n]),
                "wq": _prep_w(W_query, cols),
                "wk": _prep_w(W_key, cols),
                "wv": _prep_w(W_value, cols),
            }
        )
    res = run_bass_kernel_spmd(nc, in_maps, core_ids=list(range(8)), trace=trace)
    out = np.empty((N, T, D), dtype=np.float32)
    for c in range(8):
        n, g = c // 2, c % 2
        full = res.results[c]["oT65"]  # [260, 2048]
        for hp in range(HPC):
            blk = full[hp * OROW : (hp + 1) * OROW]
            out[n, :, g * GC + hp * DH : g * GC + (hp + 1) * DH] = (
                blk[0:DH] / blk[DH : DH + 1]
            ).T
    return out, res


def kernel(query, key, W_query, W_key, W_value):
    out, _ = run(query, key, W_query, W_key, W_value, trace=False)
    return out
